# revision 5
# baseline (speedup 1.0000x reference)
"""Mamba BasicBlock kernel for 8 Trainium2 NeuronCores (v2).

Sharding: 2 batches x 4 channel-slices (D_INNER 1536 -> 4 slices of 384).
Core c = b*4 + j handles batch b, channels [j*384,(j+1)*384), full L=2048.
Collectives: ONE AllReduce of the x_proj partial [80,2048] per 4-core
group and TWO ReduceScatters of out_proj partials over the d_model axis
(each core keeps a 192-row dm-slice of the full-L hidden output).

Per-core pipeline, fp16 compute off the PE-accumulate path:
  LN (bn_stats fp32) -> xn fp16 -> DRAM -> DMA-xbar transpose -> xnT
  -> in_proj x-half (fp16 matmul) -> causal conv (tensor_scalar@4x +
  TT@2x) + SiLU -> x_proj (fp16 matmul) -> AllReduce -> delta =
  softplus via exp+ln -> selective scan with per-n pipelining:
  dA_n = exp(A_n delta) on Act, dbx/yp split DVE(TT@2x)/Pool(gpsimd TT),
  tensor_tensor_scan on DVE (fp32 carry columns across 512-token
  blocks), fp16 tree-reduce over states -> finalize + out_proj emitted
  inline per sequence-half so the tail overlaps the scan; in_proj
  z-half, the residual add (gpsimd), and weight loads are scheduled
  into the scan phase / AllReduce bubble.

Measured: hidden absmax rel-err 9.7e-4 vs fp32 reference (residual
exact); instruction-cost-model exec estimate ~435 us/core.
"""

import os
import sys

sys.path.insert(0, "/opt/trn_rl_repo")

import numpy as np
from contextlib import ExitStack

import concourse.bass as bass
import concourse.bacc as bacc
import concourse.mybir as mybir
import concourse.tile as tile
from concourse.bass_utils import run_bass_kernel_spmd

F = mybir.dt.float32
H = mybir.dt.float16
AF = mybir.ActivationFunctionType
OP = mybir.AluOpType

B, L, DM = 2, 2048, 768
DI, DS, DC, DTR = 1536, 16, 4, 48
SL = 384          # channel slice per core
NJ = 3            # d-tiles of 128 per core
TB = 512          # scan t-block
NBLK = L // TB
NH = 8            # states per half-unit
NCORES = 8
GROUPS = [[0, 1, 2, 3], [4, 5, 6, 7]]
LN_EPS = 1e-5
TOK = L // 4      # token slice per core for outputs

# n (1-based state index) computed directly on Act as exp(a_n * delta);
# the rest come from fp16 products dA_n = dA_i * dA_j (needs a_n additive,
# which holds for the reference A = -arange(1,17); checked host-side).
ACT_NS = frozenset(range(1, 17))
POOL_SCAN_NS = frozenset()
POOL_YP_NS = frozenset((1, 3, 5, 7, 9, 11, 13))
POOL_DBX_NS = frozenset((2, 4, 6, 8, 10, 12, 14, 16))
# product decomposition for non-Act ns
PROD_PAIR = {n: (n // 2, n - n // 2) for n in range(2, DS + 1)}

_CACHE = {}


def _build(single=False):
    key = "nc1" if single else "nc"
    if key in _CACHE:
        return _CACHE[key]

    nc = bacc.Bacc("TRN2", target_bir_lowering=False, debug=False,
                   num_devices=1 if single else NCORES)

    # ---------------- I/O ----------------
    x_b = nc.dram_tensor("x_b", [L, DM], F, kind="ExternalInput").ap()
    res_x = nc.dram_tensor("res_x", [TOK, DM], F, kind="ExternalInput").ap()
    res_in = nc.dram_tensor("res_in", [TOK, DM], F, kind="ExternalInput").ap()
    W_inT = nc.dram_tensor("W_inT", [DM, 2 * SL], H, kind="ExternalInput").ap()
    bias_in = nc.dram_tensor("bias_in", [2 * SL], F, kind="ExternalInput").ap()
    WxT = nc.dram_tensor("WxT", [SL, 80], H, kind="ExternalInput").ap()
    WdtT = nc.dram_tensor("WdtT", [DTR, SL], H, kind="ExternalInput").ap()
    bdt = nc.dram_tensor("bdt", [SL], F, kind="ExternalInput").ap()
    Acols = nc.dram_tensor("Acols", [SL, DS], F, kind="ExternalInput").ap()
    convw = nc.dram_tensor("convw", [SL, DC], F, kind="ExternalInput").ap()
    convb = nc.dram_tensor("convb", [SL], F, kind="ExternalInput").ap()
    Dskip = nc.dram_tensor("Dskip", [SL], F, kind="ExternalInput").ap()
    WoT = nc.dram_tensor("WoT", [SL, DM], H, kind="ExternalInput").ap()
    hid_out = nc.dram_tensor("hid_out", [DM // 4, L], F, kind="ExternalOutput").ap()
    res_out = nc.dram_tensor("res_out", [TOK, DM], F, kind="ExternalOutput").ap()

    with tile.TileContext(nc, trace_sim=False) as tc, ExitStack() as top:
        dram = top.enter_context(tc.tile_pool(name="dram", bufs=1, space="DRAM"))
        xn_dram = dram.tile([L, DM], H)
        proj_part = dram.tile([80, L], F)
        proj_sum = dram.tile([80, L], F)
        op_part = [dram.tile([DM, L // 2], F, name=f"op_part{i}")
                   for i in range(2)]
        op_rs = [dram.tile([DM // 4, L // 2], F, name=f"op_rs{i}")
                 for i in range(2)]

        const = top.enter_context(tc.tile_pool(name="const", bufs=1))
        bias_sb = const.tile([128, 6], F)     # col m: bias_in[m*128+p]
        nc.sync.dma_start(bias_sb[:], bias_in.rearrange("(m p) -> p m", p=128))
        acol_sb = const.tile([128, NJ * DS], F)  # col j*16+n: A[j*128+p, n]
        nc.sync.dma_start(acol_sb[:].rearrange("p (j n) -> p j n", j=NJ),
                          Acols.rearrange("(j p) n -> p j n", p=128))
        convw_sb = const.tile([128, NJ * DC], F)
        nc.sync.dma_start(convw_sb[:].rearrange("p (j k) -> p j k", j=NJ),
                          convw.rearrange("(j p) k -> p j k", p=128))
        convb_sb = const.tile([128, NJ], F)
        nc.sync.dma_start(convb_sb[:], convb.rearrange("(j p) -> p j", p=128))
        dskip_sb = const.tile([128, NJ], F)
        nc.sync.dma_start(dskip_sb[:], Dskip.rearrange("(j p) -> p j", p=128))
        bdt_sb = const.tile([128, NJ], F)
        nc.sync.dma_start(bdt_sb[:], bdt.rearrange("(j p) -> p j", p=128))

        persist = top.enter_context(tc.tile_pool(name="persist", bufs=1))
        xc_sb = [persist.tile([128, L], H, tag=f"xc{j}", name=f"xc{j}")
                 for j in range(NJ)]
        z_sb = [persist.tile([128, L], H, tag=f"z{j}", name=f"z{j}")
                for j in range(NJ)]
        delta_sb = [persist.tile([128, L], H, tag=f"dl{j}", name=f"dl{j}")
                    for j in range(NJ)]
        y_sb = [persist.tile([128, L], H, tag=f"y{j}", name=f"y{j}")
                for j in range(NJ)]
        carry = [persist.tile([128, DS], F, tag=f"cr{j}", name=f"cr{j}")
                 for j in range(NJ)]

        # ============ PHASE A: LN -> xnT -> in_proj -> conv ============
        xnTp = top.enter_context(tc.tile_pool(name="xnT", bufs=1))
        with tc.tile_pool(name="xpad", bufs=1) as xpp:
            x_pad = [xpp.tile([128, L + DC - 1], H, tag=f"xp{j}", name=f"xp{j}")
                     for j in range(NJ)]
            for j in range(NJ):
                nc.vector.memset(x_pad[j][:, 0:DC - 1], 0.0)

            with tc.tile_pool(name="ln", bufs=6) as lp:
                for tt in range(L // 128):
                    xt = lp.tile([128, DM], F, tag="xt")
                    nc.sync.dma_start(xt[:], x_b[bass.ts(tt, 128), :])
                    st6 = lp.tile([128, 2 * 6], F, tag="st6")
                    nc.vector.bn_stats(st6[:, 0:6], xt[:, 0:DM // 2])
                    nc.vector.bn_stats(st6[:, 6:12], xt[:, DM // 2:DM])
                    mv = lp.tile([128, 2], F, tag="mv")
                    nc.vector.bn_aggr(mv[:], st6[:])
                    vs = lp.tile([128, 1], F, tag="vs")
                    nc.vector.tensor_scalar(out=vs[:], in0=mv[:, 1:2],
                                            scalar1=1.0, scalar2=LN_EPS,
                                            op0=OP.mult, op1=OP.add)
                    sq = lp.tile([128, 1], F, tag="sq")
                    nc.scalar.activation(sq[:], vs[:], AF.Sqrt)
                    rstd = lp.tile([128, 1], F, tag="rstd")
                    nc.vector.reciprocal(rstd[:], sq[:])
                    nmr = lp.tile([128, 1], F, tag="nmr")
                    nc.vector.scalar_tensor_tensor(
                        out=nmr[:], in0=mv[:, 0:1], scalar=-1.0,
                        in1=rstd[:], op0=OP.mult, op1=OP.mult)
                    xn = lp.tile([128, DM], H, tag="xn")
                    nc.scalar.activation(xn[:], xt[:], AF.Identity,
                                         scale=rstd[:], bias=nmr[:])
                    nc.sync.dma_start(xn_dram[bass.ts(tt, 128), :], xn[:])

            with tc.tile_pool(name="wts", bufs=1) as wp, \
                 tc.tile_pool(name="ippsum", bufs=6, space="PSUM") as ipp:
                xnT = [xnTp.tile([128, L], H, tag=f"xnT{k}", name=f"xnT{k}")
                       for k in range(6)]
                for k in range(6):
                    nc.sync.dma_start_transpose(
                        xnT[k][:], xn_dram[:, bass.ts(k, 128)])

                winT_sb = [wp.tile([128, SL], H, tag=f"wi{k}", name=f"wi{k}")
                           for k in range(6)]
                for k in range(6):
                    nc.sync.dma_start(winT_sb[k][:],
                                      W_inT[bass.ts(k, 128), 0:SL])
                for m in range(3):
                    for nb in range(4):
                        ps = ipp.tile([128, TB], F)
                        for k in range(6):
                            nc.tensor.matmul(ps[:],
                                             winT_sb[k][:, bass.ts(m, 128)],
                                             xnT[k][:, bass.ts(nb, TB)],
                                             start=(k == 0), stop=(k == 5))
                        nc.scalar.activation(
                            x_pad[m][:, DC - 1 + nb * TB:
                                     DC - 1 + (nb + 1) * TB],
                            ps[:], AF.Identity, bias=bias_sb[:, m:m + 1])

            # ---- conv (STT@4x) + silu ----
            with tc.tile_pool(name="conv", bufs=4) as cp:
                for j in range(NJ):
                    terms = []
                    for k in range(DC):
                        ak = cp.tile([128, L], H, tag="cv")
                        nc.vector.tensor_scalar(
                            out=ak[:], in0=x_pad[j][:, k:k + L],
                            scalar1=convw_sb[:, j * DC + k:j * DC + k + 1],
                            scalar2=None, op0=OP.mult)
                        terms.append(ak)
                    s0 = cp.tile([128, L], H, tag="cs")
                    nc.vector.tensor_add(s0[:], terms[0][:], terms[1][:])
                    s1 = cp.tile([128, L], H, tag="cs")
                    nc.vector.tensor_add(s1[:], terms[2][:], terms[3][:])
                    s2 = cp.tile([128, L], H, tag="cs")
                    nc.vector.tensor_add(s2[:], s0[:], s1[:])
                    nc.scalar.activation(xc_sb[j][:], s2[:], AF.Silu,
                                         bias=convb_sb[:, j:j + 1])

        # ============ PHASE B: x_proj -> AllReduce -> delta ============
        with tc.tile_pool(name="xproj", bufs=1) as xpr, \
             tc.tile_pool(name="xpps", bufs=2, space="PSUM") as xps:
            wxT_sb = [xpr.tile([128, 80], H, tag=f"wx{j}", name=f"wx{j}")
                      for j in range(NJ)]
            for j in range(NJ):
                nc.sync.dma_start(wxT_sb[j][:], WxT[bass.ts(j, 128), :])
            pp = xpr.tile([80, L], F, tag="pp")
            for nb in range(4):
                ps = xps.tile([80, TB], F)
                for j in range(NJ):
                    nc.tensor.matmul(ps[:], wxT_sb[j][:],
                                     xc_sb[j][:, bass.ts(nb, TB)],
                                     start=(j == 0), stop=(j == NJ - 1))
                nc.scalar.copy(pp[:, bass.ts(nb, TB)], ps[:])
            nc.sync.dma_start(proj_part[:, :], pp[:])
            if single:
                nc.sync.dma_start(proj_sum[:, :], proj_part[:, :])
            else:
                nc.gpsimd.collective_compute(
                    "AllReduce", OP.add, replica_groups=GROUPS,
                    ins=[proj_part[:, :].opt()],
                    outs=[proj_sum[:, :].opt()])


        # delta = softplus(W_dt @ dt + b_dt) via exp+ln, fp16 out
        with tc.tile_pool(name="dt", bufs=1) as dp, \
             tc.tile_pool(name="dtps", bufs=4, space="PSUM") as dps:
            dt32 = dp.tile([DTR, L], F, tag="dt32")
            nc.sync.dma_start(dt32[:], proj_sum[0:DTR, :])
            dtT_sb = dp.tile([DTR, L], H, tag="dtT")
            nc.scalar.copy(dtT_sb[:], dt32[:])
            wdtT_sb = dp.tile([DTR, SL], H, tag="wdt")
            nc.sync.dma_start(wdtT_sb[:], WdtT)
            for j in range(NJ):
                et = dp.tile([128, L], F, tag=f"et{j}", name=f"et{j}")
                for nb in range(4):
                    ps = dps.tile([128, TB], F)
                    nc.tensor.matmul(ps[:], wdtT_sb[:, bass.ts(j, 128)],
                                     dtT_sb[:, bass.ts(nb, TB)],
                                     start=True, stop=True)
                    nc.scalar.activation(et[:, bass.ts(nb, TB)], ps[:], AF.Exp,
                                         bias=bdt_sb[:, j:j + 1])
                nc.scalar.activation(delta_sb[j][:], et[:], AF.Ln, bias=1.0)

        # ============ PHASE C: selective scan ============
        # per (blk, j, half): dA [128,(8,TB)] fp16 (Act exp for n in ACT_NS,
        # fp16 products otherwise), dbx = (delta*xc)*B via STT@4x, 8
        # tensor_tensor_scans @4x, yp = h*C STT@4x (in-place over dbx),
        # tree-reduce into y_sb.
        bc16 = dram.tile([2 * DS, L], H, name="bc16d")
        nc.gpsimd.dma_start(bc16[:, :], proj_sum[DTR:DTR + 2 * DS, :])

        opw = top.enter_context(tc.tile_pool(name="opw", bufs=1))
        woT_sb = [opw.tile([128, DM], H, tag=f"wo{j}", name=f"wo{j}")
                  for j in range(NJ)]
        for j in range(NJ):
            nc.sync.dma_start(woT_sb[j][:], WoT[bass.ts(j, 128), :])
        winT_z = [opw.tile([128, SL], H, tag=f"wiz{k}", name=f"wiz{k}")
                  for k in range(6)]
        for k in range(6):
            nc.sync.dma_start(winT_z[k][:], W_inT[bass.ts(k, 128), SL:])

        def emit_tail(ghalf, fp, op_, ops):
            HL = L // 2
            DQ = DM // 4
            gsl = slice(ghalf * HL, (ghalf + 1) * HL)
            for j in range(NJ):
                t0 = fp.tile([128, HL], H, tag="t0")
                nc.vector.tensor_scalar(
                    out=t0[:], in0=xc_sb[j][:, gsl],
                    scalar1=dskip_sb[:, j:j + 1], scalar2=None, op0=OP.mult)
                t1 = fp.tile([128, HL], H, tag="t1")
                nc.vector.tensor_add(t1[:], t0[:], y_sb[j][:, gsl])
                nc.vector.tensor_mul(y_sb[j][:, gsl], t1[:], z_sb[j][:, gsl])
            for m in range(6):
                ot = op_.tile([128, HL], F, tag="ot")
                for c in range(2):
                    csl = slice(ghalf * HL + c * TB,
                                ghalf * HL + (c + 1) * TB)
                    ps = ops.tile([128, TB], F)
                    for j in range(NJ):
                        nc.tensor.matmul(ps[:],
                                         woT_sb[j][:, bass.ts(m, 128)],
                                         y_sb[j][:, csl],
                                         start=(j == 0), stop=(j == NJ - 1))
                    nc.scalar.copy(ot[:, bass.ts(c, TB)], ps[:])
                nc.sync.dma_start(op_part[ghalf][bass.ts(m, 128), :], ot[:])
            if single:
                nc.sync.dma_start(op_rs[ghalf][:, :], op_part[ghalf][0:DQ, :])
            else:
                nc.gpsimd.collective_compute(
                    "ReduceScatter", OP.add, replica_groups=GROUPS,
                    ins=[op_part[ghalf][:, :].opt()],
                    outs=[op_rs[ghalf][:, :].opt()])
            nc.sync.dma_start(hid_out[:, gsl], op_rs[ghalf][:, :])

        with tc.tile_pool(name="fin", bufs=2) as finp, \
             tc.tile_pool(name="oproj", bufs=2) as op_, \
             tc.tile_pool(name="opps", bufs=4, space="PSUM") as ops, \
             tc.tile_pool(name="brep", bufs=2) as bp, \
             tc.tile_pool(name="sdA", bufs=3) as adp, \
             tc.tile_pool(name="sdbx", bufs=2) as dbp, \
             tc.tile_pool(name="sh", bufs=2) as hp, \
             tc.tile_pool(name="su", bufs=2) as up, \
             tc.tile_pool(name="syt", bufs=1) as ytp:
            for blk in range(NBLK):
                tsl = slice(blk * TB, (blk + 1) * TB)
                reps = []
                for half in range(2):
                    b_rep = bp.tile([128, NH * TB], H, tag="brep")
                    c_rep = bp.tile([128, NH * TB], H, tag="crep")
                    nc.sync.dma_start(
                        b_rep[:].rearrange("p (n t) -> p n t", n=NH),
                        bc16[half * NH:(half + 1) * NH, tsl]
                        .unsqueeze(0).broadcast_to([128, NH, TB]))
                    nc.sync.dma_start(
                        c_rep[:].rearrange("p (n t) -> p n t", n=NH),
                        bc16[DS + half * NH:DS + (half + 1) * NH, tsl]
                        .unsqueeze(0).broadcast_to([128, NH, TB]))
                    reps.append((b_rep, c_rep))
                for j in range(NJ):
                    dsl = delta_sb[j][:, tsl]
                    ub = up.tile([128, TB], H, tag="ub")
                    nc.vector.tensor_mul(ub[:], dsl, xc_sb[j][:, tsl])
                    dA_half = {}
                    for half in range(2):
                        b_rep, c_rep = reps[half]
                        dA = adp.tile([128, NH * TB], H, tag="dA")
                        dA_half[half] = dA
                        dbx = dbp.tile([128, NH * TB], H, tag="dbx")
                        hs = hp.tile([128, NH * TB], H, tag="h")
                        # per-n pipeline: Act exp -> DVE dbx -> scan -> DVE yp
                        for i in range(NH):
                            n1 = half * NH + i + 1
                            seg = bass.ts(i, TB)
                            if n1 in ACT_NS:
                                nc.scalar.activation(
                                    dA[:, seg], dsl, AF.Exp,
                                    scale=acol_sb[:, j * DS + n1 - 1:
                                                  j * DS + n1])
                            else:
                                pa, pb = PROD_PAIR[n1]
                                sa = dA_half[(pa - 1) // NH][
                                    :, bass.ts((pa - 1) % NH, TB)]
                                sb_ = dA_half[(pb - 1) // NH][
                                    :, bass.ts((pb - 1) % NH, TB)]
                                nc.vector.tensor_mul(dA[:, seg], sa, sb_)
                            deng = (nc.gpsimd if n1 in POOL_DBX_NS
                                    else nc.vector)
                            deng.tensor_tensor(
                                out=dbx[:, seg], in0=ub[:],
                                in1=b_rep[:, seg], op=OP.mult)
                            n0 = half * NH + i
                            eng = (nc.gpsimd if n1 in POOL_SCAN_NS
                                   else nc.vector)
                            eng.tensor_tensor_scan(
                                out=hs[:, seg], data0=dA[:, seg],
                                data1=dbx[:, seg],
                                initial=(0.0 if blk == 0 else
                                         carry[j][:, n0:n0 + 1]),
                                op0=OP.mult, op1=OP.add)
                            # yp overwrites dbx segment (dead after scan)
                            yeng = (nc.gpsimd if n1 in POOL_YP_NS
                                    else nc.vector)
                            yeng.tensor_tensor(
                                out=dbx[:, seg], in0=hs[:, seg],
                                in1=c_rep[:, seg], op=OP.mult)
                        if blk < NBLK - 1:
                            nc.vector.tensor_copy(
                                carry[j][:, half * NH:(half + 1) * NH],
                                hs[:].rearrange("p (n t) -> p n t", n=NH)
                                [:, :, TB - 1])
                        t2 = ytp.tile([128, 4 * TB], H, tag="t2")
                        nc.vector.tensor_add(t2[:], dbx[:, 0:4 * TB],
                                             dbx[:, 4 * TB:8 * TB])
                        t3 = ytp.tile([128, 2 * TB], H, tag="t3")
                        nc.vector.tensor_add(t3[:], t2[:, 0:2 * TB],
                                             t2[:, 2 * TB:4 * TB])
                        if half == 0:
                            nc.vector.tensor_add(y_sb[j][:, tsl], t3[:, 0:TB],
                                                 t3[:, TB:2 * TB])
                        else:
                            yt = ytp.tile([128, TB], H, tag="yt")
                            nc.vector.tensor_add(yt[:], t3[:, 0:TB],
                                                 t3[:, TB:2 * TB])
                            nc.vector.tensor_add(y_sb[j][:, tsl],
                                                 y_sb[j][:, tsl], yt[:])

                if blk == 0:
                    for m in range(3):
                        for nb in range(4):
                            ps = ops.tile([128, TB], F)
                            for k in range(6):
                                nc.tensor.matmul(
                                    ps[:], winT_z[k][:, bass.ts(m, 128)],
                                    xnT[k][:, bass.ts(nb, TB)],
                                    start=(k == 0), stop=(k == 5))
                            nc.scalar.activation(
                                z_sb[m][:, bass.ts(nb, TB)], ps[:],
                                AF.Silu, bias=bias_sb[:, m + 3:m + 4])
                    for t4 in range(TOK // 128):
                        rx = finp.tile([128, DM], F, tag="rx")
                        rr = finp.tile([128, DM], F, tag="rr")
                        nc.sync.dma_start(rx[:], res_x[bass.ts(t4, 128), :])
                        nc.sync.dma_start(rr[:], res_in[bass.ts(t4, 128), :])
                        nc.gpsimd.tensor_add(rx[:], rx[:], rr[:])
                        nc.sync.dma_start(res_out[bass.ts(t4, 128), :], rx[:])
                elif blk == 1:
                    emit_tail(0, finp, op_, ops)
                elif blk == 3:
                    emit_tail(1, finp, op_, ops)

    nc.compile()
    _CACHE[key] = nc
    return nc


def _prep_inputs(inp):
    gamma, beta = inp["ln_gamma"], inp["ln_beta"]
    W_in = inp["W_in"]
    W_in_f = W_in * gamma[None, :]
    bias_full = W_in @ beta            # [2*DI]
    A = -np.exp(inp["A_log"])          # [DI, DS]

    in_maps = []
    for c in range(NCORES):
        b, j = c // 4, c % 4
        S = slice(j * SL, (j + 1) * SL)
        rows = np.r_[j * SL:(j + 1) * SL, DI + j * SL:DI + (j + 1) * SL]
        m = {
            "x_b": inp["x"][b],
            "res_x": inp["x"][b, j * TOK:(j + 1) * TOK],
            "res_in": inp["residual"][b, j * TOK:(j + 1) * TOK],
            "W_inT": np.ascontiguousarray(W_in_f[rows].T).astype(np.float16),
            "bias_in": np.ascontiguousarray(bias_full[rows]),
            "WxT": np.ascontiguousarray(inp["W_xproj"][:, S].T).astype(np.float16),
            "WdtT": np.ascontiguousarray(inp["W_dt"][S].T).astype(np.float16),
            "bdt": np.ascontiguousarray(inp["b_dt"][S]),
            "Acols": np.ascontiguousarray(A[S]),
            "convw": np.ascontiguousarray(inp["conv_w"][S]),
            "convb": np.ascontiguousarray(inp["conv_b"][S]),
            "Dskip": np.ascontiguousarray(inp["D_skip"][S]),
            "WoT": np.ascontiguousarray(inp["W_out"][:, S].T).astype(np.float16),
        }
        in_maps.append(m)
    return in_maps


def _assemble(results):
    hidden = np.empty((B, L, DM), np.float32)
    residual = np.empty((B, L, DM), np.float32)
    for c in range(NCORES):
        b, j = c // 4, c % 4
        r = results[c]
        hidden[b, :, j * (DM // 4):(j + 1) * (DM // 4)] = r["hid_out"].T
        residual[b, j * TOK:(j + 1) * TOK] = r["res_out"]
    return hidden, residual


def kernel(**inputs):
    inp = {k: np.ascontiguousarray(np.asarray(v, dtype=np.float32))
           for k, v in inputs.items()}
    nc = _build()
    in_maps = _prep_inputs(inp)
    res = run_bass_kernel_spmd(nc, in_maps, list(range(NCORES)))
    return _assemble(res.results)
